# revision 19
# baseline (speedup 1.0000x reference)
"""Trainium2 Bass kernel for paged-KV attention block (QKV proj + RoPE +
paged causal attention + o_proj), tensor-parallel over heads across 8 cores.

Contract: kernel(**inputs) takes the full unsharded inputs (numpy or jax
arrays, keyed as in the reference setup_inputs) and returns the full
[B*Lq, hidden] float32 output.

Sharding (per the tensor-parallel hint):
  - W_pack sharded over heads: each core owns 4 heads of q, k, v rows.
  - KV cache and attention sharded over the same heads.
  - o_proj row-sharded; each core computes a full [T, hidden] partial (fp16)
    and the partials are summed on the host (replaces the all-reduce).

Device schedule (v2):
  - QKV in transposed [feature, token] layout; fresh q/k land in the [d, t]
    layout scores need; v is PE-transposed back to [t, d] tiles.
  - K history pre-transposed on host to [h, b, d, kv]; V history pre-tiled
    to [h, b, p, j, d]; both DMA'd early in the QKV phase (not at attention
    start) so attention never waits on them.
  - w_o is hoisted to SBUF once for the whole kernel.
  - Scores as S^T [kv, q]; exp fused with PSUM eviction + 1/sqrt(D) scale on
    ScalarE. Causal structure exploited: fresh-kv tiles only compute the
    q >= kv columns; only the diagonal 128x128 block needs a mask multiply.
    Fresh PV/den accumulate in reverse kv order so the last (full-width)
    matmul carries the accumulation stop flag.
  - Softmax denominator: P tiles are accumulated on the (otherwise idle-ish)
    Vector engine into a per-head running sum; a single ones-vector matmul
    per head reduces it over partitions. This keeps the Tensor engine free
    of the 256 denominator matmuls and frees a PSUM bank.
  - o_proj of sequence b-1 is software-pipelined into the attention phase of
    sequence b (one [t-tile, 512-col] group every 2 attention units), filling
    the Tensor-engine bubbles that ScalarE's exp throughput would otherwise
    leave. The last sequence's o_proj runs at the end.
  - Output partials are written fp16 (host sums in fp32).
"""

import math
import os

import numpy as np

import concourse.bacc as bacc
import concourse.tile as tile
from concourse import mybir
from concourse.bass_utils import run_bass_kernel_spmd

F32 = mybir.dt.float32
BF16 = mybir.dt.bfloat16
FP16 = mybir.dt.float16

N_CORES = 8


def build_kernel(B, Lq, H, D, hidden, hist, hpc):
    """Build the SPMD single-core program. hpc = heads per core."""
    assert D == 128 and Lq == 512 and hist % 128 == 0
    Fqk = hpc * D          # per-core q (or k) feature count = 512
    F3 = 3 * Fqk           # per-core packed qkv features = 1536
    T = B * Lq
    C = hidden
    NCT = C // 128         # contraction tiles = 32
    NJH = hist // 128      # kv tiles in history = 12
    NJF = Lq // 128        # kv tiles fresh = 4
    NJ = NJH + NJF         # 16
    NOC = hidden // 512    # o_proj column chunks = 8
    NFP = (3 * hpc) // 2   # wp 2-head pair loads per seq = 6
    scale = 1.0 / math.sqrt(D)
    EXP_BIAS = -8.0
    dq = FP16              # qkv matmul dtype
    da = FP16              # attention matmul dtype
    do = FP16              # o_proj matmul dtype

    nc = bacc.Bacc("TRN2")

    hT = nc.dram_tensor("hT", [C, T], dq, kind="ExternalInput")
    wpT = nc.dram_tensor("wpT", [C, F3], dq, kind="ExternalInput")
    woT = nc.dram_tensor("woT", [Fqk, hidden], do, kind="ExternalInput")
    kTh = nc.dram_tensor("kTh", [hpc, B, D, hist], da, kind="ExternalInput")
    vh = nc.dram_tensor("vh", [hpc, B, 128, NJH, 128], da, kind="ExternalInput")
    cosT = nc.dram_tensor("cosT", [D, Lq], FP16, kind="ExternalInput")
    sinT = nc.dram_tensor("sinT", [D, Lq], FP16, kind="ExternalInput")
    RmT = nc.dram_tensor("RmT", [D, D], FP16, kind="ExternalInput")
    triT = nc.dram_tensor("triT", [128, 128], FP16, kind="ExternalInput")
    outp = nc.dram_tensor("outp", [T, hidden], FP16, kind="ExternalOutput")

    NHC = 8                # hT DMA chunks per seq
    HCT = NCT // NHC       # c-tiles per hT chunk = 4
    with tile.TileContext(nc) as tc:
        with (
            tc.tile_pool(name="const", bufs=1) as p_const,
            tc.tile_pool(name="hTp", bufs=2) as p_hT,
            tc.tile_pool(name="wpp", bufs=2) as p_wp,
            tc.tile_pool(name="qsp", bufs=2) as p_qs,
            tc.tile_pool(name="qk", bufs=2) as p_qk,
            tc.tile_pool(name="vnatp", bufs=2) as p_vnat,
            tc.tile_pool(name="attnTp", bufs=2) as p_attnT,
            tc.tile_pool(name="hist", bufs=1) as p_hist,
            tc.tile_pool(name="Pp", bufs=10) as p_p,
            tc.tile_pool(name="Pfp", bufs=3) as p_pf,
            tc.tile_pool(name="denp", bufs=2) as p_den,
            tc.tile_pool(name="smalls", bufs=2) as p_small,
            tc.tile_pool(name="oep", bufs=3) as p_oe,
            tc.tile_pool(name="ps_mm", bufs=2, space="PSUM") as ps_mm,
            tc.tile_pool(name="ps_rot", bufs=2, space="PSUM") as ps_rot,
            tc.tile_pool(name="ps_s", bufs=2, space="PSUM") as ps_s,
            tc.tile_pool(name="ps_pv", bufs=2, space="PSUM") as ps_pv,
        ):
            consts = {}

            def emit_small_consts():
                cos_sb = p_const.tile([D, Lq], FP16, tag="cos", name="cos")
                nc.sync.dma_start(out=cos_sb, in_=cosT[:, :])
                sin_sb = p_const.tile([D, Lq], FP16, tag="sin", name="sin")
                nc.sync.dma_start(out=sin_sb, in_=sinT[:, :])
                rm16 = p_const.tile([D, D], FP16, tag="rm16", name="rm16")
                nc.sync.dma_start(out=rm16, in_=RmT[:, :])
                tri = p_const.tile([128, 128], FP16, tag="tri", name="tri")
                nc.sync.dma_start(out=tri, in_=triT[:, :])
                ident_sb = p_const.tile([128, 128], F32, tag="ident", name="ident")
                from concourse.masks import make_identity

                make_identity(nc, ident_sb[:, :])
                ident16 = p_const.tile([128, 128], FP16, tag="ident16", name="ident16")
                nc.vector.tensor_copy(ident16, ident_sb)
                ones_f32 = p_const.tile([128, 1], F32, tag="ones_f32", name="ones_f32")
                nc.vector.memset(ones_f32, 1.0)
                ones_col = p_const.tile([128, 1], da, tag="ones_col", name="ones_col")
                nc.vector.tensor_copy(ones_col, ones_f32)
                ones_row = p_const.tile([1, 128], F32, tag="ones_row", name="ones_row")
                nc.vector.memset(ones_row, 1.0)
                ones_row16 = p_const.tile([1, 128], da, tag="ones_row16", name="ones_row16")
                nc.vector.tensor_copy(ones_row16, ones_row)
                ebias_sb = p_const.tile([128, 1], F32, tag="ebias", name="ebias")
                nc.vector.memset(ebias_sb, EXP_BIAS)
                consts.update(
                    cos=cos_sb, sin=sin_sb, rm16=rm16, tri=tri, ident16=ident16,
                    ones_col=ones_col, ones_row16=ones_row16, ebias=ebias_sb,
                )

            def emit_wo_load():
                wo_sb = p_const.tile([128, hpc, hidden], do, tag="wo", name="wo")
                nc.sync.dma_start(
                    out=wo_sb,
                    in_=woT[:, :].rearrange("(jt p) o -> p jt o", p=128),
                )
                consts["wo"] = wo_sb

            def load_wp_chunk(fp, wh, qh):
                t = p_wp.tile(
                    [128, NCT // 4, 256], dq,
                    tag=f"wp{wh}{qh}", name=f"wp{wh}{qh}",
                )
                r0 = wh * (C // 2) + qh * (C // 4)
                nc.sync.dma_start(
                    out=t,
                    in_=wpT[
                        r0 : r0 + C // 4,
                        fp * 256 : (fp + 1) * 256,
                    ].rearrange("(ct p) f -> p ct f", p=128),
                )
                return t

            def load_wp_pair(fp):
                # 4 chunk tiles per pair (2 C-halves x 2 ct-halves) so the
                # first matmul only gates on a quarter of the pair's bytes.
                return [
                    [load_wp_chunk(fp, wh, qh) for qh in range(2)]
                    for wh in range(2)
                ]

            # per-seq state kept across emit stages
            seq_state = {}
            hist_tiles = {}

            def load_hist(b, h):
                kt = p_hist.tile([128, hist], da, tag=f"kth{h}", name=f"kth{h}")
                nc.sync.dma_start(out=kt, in_=kTh[h, b])
                vt = p_hist.tile([128, NJH, 128], da, tag=f"vh{h}", name=f"vh{h}")
                nc.sync.dma_start(out=vt, in_=vh[h, b])
                hist_tiles[(b, h)] = (kt, vt)

            def emit_qkv(b):
                st = {}
                seq_state[b] = st

                # Interleave the wp0 chunks with the hT chunks in transfer
                # order so the first f-tile's matmul chain starts after ~1MB
                # and chases the DMA stream instead of waiting for all of it.
                def load_hT(cc):
                    t = p_hT.tile([128, HCT, Lq], dq, tag=f"hT{cc}", name=f"hT{cc}")
                    nc.sync.dma_start(
                        out=t,
                        in_=hT[
                            cc * HCT * 128 : (cc + 1) * HCT * 128,
                            b * Lq : (b + 1) * Lq,
                        ].rearrange("(ct p) t -> p ct t", p=128),
                    )
                    return t

                wp0 = [[None, None], [None, None]]
                hT_c = [None] * NHC
                wp0[0][0] = load_wp_chunk(0, 0, 0)
                hT_c[0] = load_hT(0)
                wp0[0][1] = load_wp_chunk(0, 0, 1)
                hT_c[1] = load_hT(1)
                hT_c[2] = load_hT(2)
                wp0[1][0] = load_wp_chunk(0, 1, 0)
                hT_c[3] = load_hT(3)
                if b == 0:
                    emit_small_consts()
                wp0[1][1] = load_wp_chunk(0, 1, 1)
                for cc in range(4, NHC):
                    hT_c[cc] = load_hT(cc)

                qrot = [None] * hpc
                krot = [None] * hpc
                vnat = [
                    p_vnat.tile([128, Fqk], da, tag=f"vnat{i}", name=f"vnat{i}")
                    for i in range(NJF)
                ]
                st.update(qrot=qrot, krot=krot, vnat=vnat)

                # epilogue of f-tile ft (RoPE or v-transposes), deferred by
                # one f-tile so the PE never stalls on the ScalarE eviction.
                def qkv_epilogue(ft, qs):
                    if ft < 2 * hpc:
                        pr = ps_rot.tile([128, Lq], F32, tag="rot", name="rot")
                        nc.tensor.matmul(pr, consts["rm16"], qs, start=True, stop=True)
                        tag = f"qrot{ft}" if ft < hpc else f"krot{ft - hpc}"
                        tmp1 = p_qs.tile([128, Lq], FP16, tag="tmp1", name="tmp1")
                        nc.vector.tensor_mul(tmp1, qs, consts["cos"])
                        tmp = p_qs.tile([128, Lq], FP16, tag="tmp", name="tmp")
                        nc.vector.tensor_mul(tmp, pr, consts["sin"])
                        dst = p_qk.tile([128, Lq], da, tag=tag)
                        nc.vector.tensor_add(dst, tmp1, tmp)
                        if ft < hpc:
                            qrot[ft] = dst
                        else:
                            krot[ft - hpc] = dst
                    else:
                        fv = ft - 2 * hpc
                        for tsub in range(NJF):
                            pt = ps_rot.tile([128, Lq], FP16, tag="rot", name="rot")
                            nc.tensor.transpose(
                                pt[:, 0:128],
                                qs[:, tsub * 128 : (tsub + 1) * 128],
                                consts["ident16"][:, :],
                            )
                            nc.vector.tensor_copy(
                                vnat[tsub][:, fv * 128 : (fv + 1) * 128],
                                pt[:, 0:128],
                            )

                st["epilogue"] = qkv_epilogue

                pending = None
                for fp in range(NFP):
                    wp_h = wp0 if fp == 0 else load_wp_pair(fp)
                    for sub in range(2):
                        ft = 2 * fp + sub
                        ps = ps_mm.tile([128, Lq], F32, tag="mm", name="mm")
                        for ct in range(NCT):
                            nc.tensor.matmul(
                                ps,
                                wp_h[ct // (NCT // 2)][(ct % (NCT // 2)) // (NCT // 4)][
                                    :, ct % (NCT // 4), sub * 128 : (sub + 1) * 128
                                ],
                                hT_c[ct // HCT][:, ct % HCT, :],
                                start=(ct == 0),
                                stop=(ct == NCT - 1),
                            )
                        qs = p_qs.tile([128, Lq], FP16, tag="qs", name="qs")
                        nc.scalar.copy(qs, ps)
                        if pending is not None:
                            qkv_epilogue(*pending)
                        pending = (ft, qs)
                st["pending"] = pending

                # history K/V for this sequence. For b=0 issue now (behind
                # the wp/hT stream); for b>0 the loads were already issued
                # inside attn(b-1) as each head's tiles freed up.
                if b == 0:
                    for h in range(hpc):
                        load_hist(0, h)
                kth_t = [None] * hpc
                vh_tt = [None] * hpc
                for h in range(hpc):
                    kth_t[h], vh_tt[h] = hist_tiles.pop((b, h))
                st.update(kth=kth_t, vh=vh_tt)
                if b == 0:
                    emit_wo_load()

            def oproj_steps(b):
                """Generator of o_proj emission steps for sequence b.
                Each step: one (oc, tsub) group = hpc accumulating matmuls +
                fp16 eviction + output DMA. 32 steps total."""
                attnT = seq_state[b]["attnT"]
                for oc in range(NOC):
                    for tsub in range(NJF):
                        po = ps_mm.tile([128, 512], F32, tag="mm", name="mm")
                        for j in range(hpc):
                            nc.tensor.matmul(
                                po,
                                attnT[j][:, tsub * 128 : (tsub + 1) * 128],
                                consts["wo"][:, j, oc * 512 : (oc + 1) * 512],
                                start=(j == 0),
                                stop=(j == hpc - 1),
                            )
                        oe = p_oe.tile([128, 512], FP16, tag="oe", name="oe")
                        nc.vector.tensor_copy(oe, po)
                        row = b * Lq + tsub * 128
                        nc.sync.dma_start(
                            out=outp[row : row + 128, oc * 512 : (oc + 1) * 512],
                            in_=oe,
                        )
                        yield

            def emit_attn(b, op_iter):
                """Attention for sequence b, with o_proj steps of sequence
                b-1 (op_iter) interleaved every 2 units."""
                st = seq_state[b]
                qrot, krot, vnat = st["qrot"], st["krot"], st["vnat"]
                kth_t, vh_tt = st["kth"], st["vh"]
                attnT = [None] * hpc
                st["attnT"] = attnT
                P_t = {}
                pv_ps = {}
                den_acc = {}
                den_ps = {}
                actions = []   # (due_unit, fn), emitted after S/exp of a unit
                unit = 0
                pending_qkv = [st["pending"]]

                # unit order per head: history j=0..NJH-1 full width, then
                # fresh kv blocks in REVERSE order (jj=NJF-1 .. 0) with
                # partial q widths so the last fresh matmul is full-width and
                # carries the accumulation stop flag.
                def unit_j(u):
                    if u < NJH:
                        return u, 0           # j, q-offset
                    jj = NJF - 1 - (u - NJH)  # NJF-1 .. 0
                    return NJH + jj, jj * 128

                def emit_pv(h, u):
                    def fn():
                        j, qoff = unit_j(u)
                        pvh = pv_ps[h]
                        if j < NJH:
                            v_lhsT = vh_tt[h][:, j, :]
                        else:
                            v_lhsT = vnat[j - NJH][:, h * 128 : (h + 1) * 128]
                        P = P_t.pop((h, u))
                        nc.tensor.matmul(
                            pvh[:, qoff:Lq], v_lhsT, P[:, qoff:Lq],
                            start=(u == 0), stop=(u == NJ - 1),
                        )
                    return fn

                def emit_den_add(h, u):
                    def fn():
                        j, qoff = unit_j(u)
                        A = den_acc[h]
                        P = P_t[(h, u)]
                        if u == 0:
                            nc.vector.tensor_copy(A, P)
                        else:
                            nc.vector.tensor_add(
                                A[:, qoff:Lq], A[:, qoff:Lq], P[:, qoff:Lq]
                            )
                    return fn

                def emit_den_mm(h):
                    def fn():
                        dps = ps_rot.tile([128, Lq], F32, tag="rot", name="den")
                        den_ps[h] = dps
                        nc.tensor.matmul(
                            dps[0:1, :], consts["ones_col"], den_acc[h],
                            start=True, stop=True,
                        )
                        # this head's history tiles are fully consumed (last
                        # PV was emitted at due unit+6 < unit+7): start the
                        # next sequence's loads into the freed buffers.
                        if b + 1 < B:
                            load_hist(b + 1, h)
                    return fn

                def emit_norm(h):
                    def fn():
                        pvh = pv_ps[h]
                        recf = p_small.tile([1, Lq], F32, tag="recf", name="recf")
                        nc.vector.reciprocal_approx_fast(
                            out=recf, in_=den_ps[h][0:1, :]
                        )
                        recip = p_small.tile([1, Lq], da, tag="recip", name="recip")
                        nc.vector.tensor_copy(recip, recf)
                        bc = ps_rot.tile([128, Lq], F32, tag="rot", name="bc")
                        nc.tensor.matmul(
                            bc, consts["ones_row16"], recip, start=True, stop=True
                        )
                        bcs = p_small.tile([128, Lq], da, tag="bcs", name="bcs")
                        nc.vector.tensor_copy(bcs, bc)
                        at = p_attnT.tile(
                            [128, Lq], do, tag=f"attnT{h}", name=f"attnT{h}"
                        )
                        nc.vector.tensor_mul(at, pvh, bcs)
                        attnT[h] = at
                    return fn

                for h in range(hpc):
                    pv_ps[h] = ps_pv.tile([128, Lq], F32, tag="pv", name="pv")
                    den_acc[h] = p_den.tile([128, Lq], da, tag="A", name="A")
                    for u in range(NJ):
                        j, qoff = unit_j(u)
                        w = Lq - qoff
                        sp = ps_s.tile([128, Lq], F32, tag="sps", name="sps")
                        if j < NJH:
                            k_lhsT = kth_t[h][:, j * 128 : (j + 1) * 128]
                        else:
                            jj = j - NJH
                            k_lhsT = krot[h][:, jj * 128 : (jj + 1) * 128]
                        nc.tensor.matmul(
                            sp[:, qoff:Lq], k_lhsT, qrot[h][:, qoff:Lq],
                            start=True, stop=True,
                        )
                        P = p_p.tile([128, Lq], da, tag="P", name="P")
                        if j < NJH:
                            nc.scalar.activation(
                                P, sp, mybir.ActivationFunctionType.Exp,
                                scale=scale, bias=consts["ebias"][:, :],
                            )
                        else:
                            # diagonal block: exp then triangular mask
                            Pf = p_pf.tile([128, 128], da, tag="Pf", name="Pf")
                            nc.scalar.activation(
                                Pf, sp[:, qoff : qoff + 128],
                                mybir.ActivationFunctionType.Exp,
                                scale=scale, bias=consts["ebias"][:, :],
                            )
                            nc.vector.tensor_mul(
                                P[:, qoff : qoff + 128], Pf, consts["tri"]
                            )
                            if qoff + 128 < Lq:
                                nc.scalar.activation(
                                    P[:, qoff + 128 : Lq], sp[:, qoff + 128 : Lq],
                                    mybir.ActivationFunctionType.Exp,
                                    scale=scale, bias=consts["ebias"][:, :],
                                )
                        P_t[(h, u)] = P
                        if pending_qkv and unit == 1:
                            st["epilogue"](*pending_qkv.pop())
                        actions.append((unit + 2, emit_den_add(h, u)))
                        actions.append(
                            (unit + (6 if j >= NJH else 3), emit_pv(h, u))
                        )
                        if u == NJ - 1:
                            actions.append((unit + 11, emit_den_mm(h)))
                            actions.append((unit + 13, emit_norm(h)))
                        unit += 1
                        while actions and actions[0][0] <= unit:
                            actions.pop(0)[1]()
                        if op_iter is not None and unit % 2 == 0:
                            next(op_iter, None)
                while actions:
                    actions.pop(0)[1]()
                if op_iter is not None:
                    for _ in op_iter:
                        pass

            for b in range(B):
                emit_qkv(b)
                emit_attn(b, oproj_steps(b - 1) if b > 0 else None)
            for _ in oproj_steps(B - 1):
                pass
    nc.compile()
    return nc


def _np_dt(d):
    return mybir.dt.np(d)


def prepare_host_inputs(inputs):
    """Shard + relayout the full inputs into 8 per-core input maps."""
    hidden_states = np.ascontiguousarray(
        np.asarray(inputs["hidden_states"], np.float32)
    )
    w_pack = np.asarray(inputs["w_pack"], np.float32)
    w_o = np.asarray(inputs["w_o"], np.float32)
    k_cache = np.asarray(inputs["k_cache"], np.float32)
    v_cache = np.asarray(inputs["v_cache"], np.float32)
    block_offsets = np.asarray(inputs["block_offsets"])
    hist = int(inputs["history_len"])
    Lq = int(inputs["q_len"])
    bs = int(inputs["block_size"])

    B, nblk = block_offsets.shape
    H, D = k_cache.shape[2], k_cache.shape[3]
    hidden = H * D
    T = B * Lq
    assert hidden_states.shape == (T, hidden)
    assert hist % bs == 0 and Lq % bs == 0 and hist % 128 == 0
    hpc = H // N_CORES

    f16 = np.float16

    # shared tensors
    hT = np.ascontiguousarray(hidden_states.T).astype(f16)

    pos = hist + np.arange(Lq, dtype=np.float64)
    inv_freq = 1.0 / (10000.0 ** (np.arange(0, D, 2, dtype=np.float64) / D))
    ang = pos[None, :] * inv_freq[np.arange(D) % (D // 2), None]  # [D, Lq]
    cosT = np.ascontiguousarray(np.cos(ang)).astype(f16)
    sinT = np.ascontiguousarray(np.sin(ang)).astype(f16)

    Rm = np.zeros((D, D), np.float32)
    half = D // 2
    for d in range(half):
        Rm[d + half, d] = -1.0
    for d in range(half, D):
        Rm[d - half, d] = 1.0
    RmT = Rm.astype(f16)

    # [kv, q] diagonal-block causal mask: allow q >= kv
    triT = np.ascontiguousarray(np.triu(np.ones((128, 128), f16)))

    # paged gather of the history KV (host side = the sharding relayout)
    nhist_blk = hist // bs
    blocks_hist = block_offsets[:, :nhist_blk]
    k_hist = k_cache[blocks_hist].reshape(B, hist, H, D)
    v_hist = v_cache[blocks_hist].reshape(B, hist, H, D)
    NJH = hist // 128

    in_maps = []
    for c in range(N_CORES):
        hs = slice(c * hpc, (c + 1) * hpc)
        rows = np.concatenate(
            [
                q * hidden + np.arange(c * hpc * D, (c + 1) * hpc * D)
                for q in range(3)
            ]
        )
        wpT_c = np.ascontiguousarray(w_pack[rows].T).astype(f16)
        woT_c = np.ascontiguousarray(
            w_o[:, c * hpc * D : (c + 1) * hpc * D].T
        ).astype(f16)
        kTh_c = np.ascontiguousarray(
            k_hist[:, :, hs, :].transpose(2, 0, 3, 1)
        ).astype(f16)
        # v history pre-tiled: [h, b, p, j, d] with kv = j*128 + p
        vh_c = np.ascontiguousarray(
            v_hist[:, :, hs, :]
            .reshape(B, NJH, 128, hpc, D)
            .transpose(3, 0, 2, 1, 4)
        ).astype(f16)
        in_maps.append(
            {
                "hT": hT,
                "wpT": wpT_c,
                "woT": woT_c,
                "kTh": kTh_c,
                "vh": vh_c,
                "cosT": cosT,
                "sinT": sinT,
                "RmT": RmT,
                "triT": triT,
            }
        )
    meta = dict(B=B, Lq=Lq, H=H, D=D, hidden=hidden, hist=hist, hpc=hpc)
    return in_maps, meta


_NC_CACHE = {}


def get_nc(meta):
    key = tuple(sorted(meta.items()))
    if key not in _NC_CACHE:
        _NC_CACHE[key] = build_kernel(**meta)
    return _NC_CACHE[key]


def run(inputs, trace=False):
    in_maps, meta = prepare_host_inputs(inputs)
    nc = get_nc(meta)
    res = run_bass_kernel_spmd(nc, in_maps, list(range(N_CORES)), trace=trace)
    out = res.results[0]["outp"].astype(np.float32)
    for i in range(1, N_CORES):
        out += res.results[i]["outp"].astype(np.float32)
    return out, res


def kernel(**inputs):
    out, _ = run(inputs, trace=False)
    return out


# revision 29
# speedup vs baseline: 1.1833x; 1.1833x over previous
"""Trainium2 Bass kernel for paged-KV attention block (QKV proj + RoPE +
paged causal attention + o_proj), tensor-parallel over heads across 8 cores.

Contract: kernel(**inputs) takes the full unsharded inputs (numpy or jax
arrays, keyed as in the reference setup_inputs) and returns the full
[B*Lq, hidden] float32 output.

Sharding (per the tensor-parallel hint):
  - W_pack sharded over heads: each core owns 4 heads of q, k, v rows.
  - KV cache and attention sharded over the same heads.
  - o_proj row-sharded; each core computes a full [T, hidden] partial (fp16)
    and the partials are summed on the host (replaces the all-reduce).

Device schedule (v2):
  - QKV in transposed [feature, token] layout; fresh q/k land in the [d, t]
    layout scores need; v is PE-transposed back to [t, d] tiles.
  - K history pre-transposed on host to [h, b, d, kv]; V history pre-tiled
    to [h, b, p, j, d]; both DMA'd early in the QKV phase (not at attention
    start) so attention never waits on them.
  - w_o is hoisted to SBUF once for the whole kernel.
  - Scores as S^T [kv, q]; exp fused with PSUM eviction + 1/sqrt(D) scale on
    ScalarE. Causal structure exploited: fresh-kv tiles only compute the
    q >= kv columns; only the diagonal 128x128 block needs a mask multiply.
    Fresh PV/den accumulate in reverse kv order so the last (full-width)
    matmul carries the accumulation stop flag.
  - Softmax denominator: P tiles are accumulated on the (otherwise idle-ish)
    Vector engine into a per-head running sum; a single ones-vector matmul
    per head reduces it over partitions. This keeps the Tensor engine free
    of the 256 denominator matmuls and frees a PSUM bank.
  - o_proj of sequence b-1 is software-pipelined into the attention phase of
    sequence b (one [t-tile, 512-col] group every 2 attention units), filling
    the Tensor-engine bubbles that ScalarE's exp throughput would otherwise
    leave. The last sequence's o_proj runs at the end.
  - Output partials are written fp16 (host sums in fp32).
"""

import math
import os

import numpy as np

import concourse.bacc as bacc
import concourse.tile as tile
from concourse import mybir
from concourse.bass_utils import run_bass_kernel_spmd

F32 = mybir.dt.float32
BF16 = mybir.dt.bfloat16
FP16 = mybir.dt.float16

N_CORES = 8


def build_kernel(B, Lq, H, D, hidden, hist, hpc):
    """Build the SPMD single-core program. hpc = heads per core."""
    assert D == 128 and Lq == 512 and hist % 128 == 0
    Fqk = hpc * D          # per-core q (or k) feature count = 512
    F3 = 3 * Fqk           # per-core packed qkv features = 1536
    T = B * Lq
    C = hidden
    NCT = C // 128         # contraction tiles = 32
    NJH = hist // 128      # kv tiles in history = 12
    NJF = Lq // 128        # kv tiles fresh = 4
    NJ = NJH + NJF         # 16
    NOC = hidden // 512    # o_proj column chunks = 8
    NFP = (3 * hpc) // 2   # wp 2-head pair loads per seq = 6
    scale = 1.0 / math.sqrt(D)
    EXP_BIAS = -8.0
    dq = FP16              # qkv matmul dtype
    da = FP16              # attention matmul dtype
    do = FP16              # o_proj matmul dtype

    nc = bacc.Bacc("TRN2")

    hT = nc.dram_tensor("hT", [C, T], dq, kind="ExternalInput")
    wpT = nc.dram_tensor("wpT", [C, F3], dq, kind="ExternalInput")
    woT = nc.dram_tensor("woT", [Fqk, hidden], do, kind="ExternalInput")
    kTh = nc.dram_tensor("kTh", [hpc, B, D, hist], da, kind="ExternalInput")
    vh = nc.dram_tensor("vh", [hpc, B, 128, NJH, 128], da, kind="ExternalInput")
    cosT = nc.dram_tensor("cosT", [D, Lq], FP16, kind="ExternalInput")
    sinT = nc.dram_tensor("sinT", [D, Lq], FP16, kind="ExternalInput")
    RmT = nc.dram_tensor("RmT", [D, D], FP16, kind="ExternalInput")
    triT = nc.dram_tensor("triT", [128, 128], FP16, kind="ExternalInput")
    outp = nc.dram_tensor("outp", [T, hidden], FP16, kind="ExternalOutput")

    NHC = 8                # hT DMA chunks per seq
    HCT = NCT // NHC       # c-tiles per hT chunk = 4
    with tile.TileContext(nc) as tc:
        with (
            tc.tile_pool(name="const", bufs=1) as p_const,
            tc.tile_pool(name="hTp", bufs=2) as p_hT,
            tc.tile_pool(name="hTp1", bufs=1) as p_hT1,
            tc.tile_pool(name="wpp", bufs=2) as p_wp,
            tc.tile_pool(name="qsp", bufs=2) as p_qs,
            tc.tile_pool(name="qk", bufs=2) as p_qk,
            tc.tile_pool(name="vnatp", bufs=2) as p_vnat,
            tc.tile_pool(name="attnTp", bufs=2) as p_attnT,
            tc.tile_pool(name="hist", bufs=1) as p_hist,
            tc.tile_pool(name="Pp", bufs=10) as p_p,
            tc.tile_pool(name="Pfp", bufs=3) as p_pf,
            tc.tile_pool(name="denp", bufs=2) as p_den,
            tc.tile_pool(name="smalls", bufs=2) as p_small,
            tc.tile_pool(name="oep", bufs=3) as p_oe,
            tc.tile_pool(name="ps_mm", bufs=2, space="PSUM") as ps_mm,
            tc.tile_pool(name="ps_rot", bufs=2, space="PSUM") as ps_rot,
            tc.tile_pool(name="ps_s", bufs=2, space="PSUM") as ps_s,
            tc.tile_pool(name="ps_pv", bufs=2, space="PSUM") as ps_pv,
        ):
            consts = {}

            def emit_small_consts():
                cos_sb = p_const.tile([D, Lq], FP16, tag="cos", name="cos")
                nc.sync.dma_start(out=cos_sb, in_=cosT[:, :])
                sin_sb = p_const.tile([D, Lq], FP16, tag="sin", name="sin")
                nc.sync.dma_start(out=sin_sb, in_=sinT[:, :])
                rm16 = p_const.tile([D, D], FP16, tag="rm16", name="rm16")
                nc.sync.dma_start(out=rm16, in_=RmT[:, :])
                tri = p_const.tile([128, 128], FP16, tag="tri", name="tri")
                nc.sync.dma_start(out=tri, in_=triT[:, :])
                ident_sb = p_const.tile([128, 128], F32, tag="ident", name="ident")
                from concourse.masks import make_identity

                make_identity(nc, ident_sb[:, :])
                ident16 = p_const.tile([128, 128], FP16, tag="ident16", name="ident16")
                nc.vector.tensor_copy(ident16, ident_sb)
                ones_f32 = p_const.tile([128, 1], F32, tag="ones_f32", name="ones_f32")
                nc.vector.memset(ones_f32, 1.0)
                ones_col = p_const.tile([128, 1], da, tag="ones_col", name="ones_col")
                nc.vector.tensor_copy(ones_col, ones_f32)
                ones_row = p_const.tile([1, 128], F32, tag="ones_row", name="ones_row")
                nc.vector.memset(ones_row, 1.0)
                ones_row16 = p_const.tile([1, 128], da, tag="ones_row16", name="ones_row16")
                nc.vector.tensor_copy(ones_row16, ones_row)
                ebias_sb = p_const.tile([128, 1], F32, tag="ebias", name="ebias")
                nc.vector.memset(ebias_sb, EXP_BIAS)
                consts.update(
                    cos=cos_sb, sin=sin_sb, rm16=rm16, tri=tri, ident16=ident16,
                    ones_col=ones_col, ones_row16=ones_row16, ebias=ebias_sb,
                )

            def emit_wo_load():
                wo_sb = p_const.tile([128, hpc, hidden], do, tag="wo", name="wo")
                nc.sync.dma_start(
                    out=wo_sb,
                    in_=woT[:, :].rearrange("(jt p) o -> p jt o", p=128),
                )
                consts["wo"] = wo_sb

            def load_wp_chunk(fp, wh, qh):
                t = p_wp.tile(
                    [128, NCT // 4, 256], dq,
                    tag=f"wp{wh}{qh}", name=f"wp{wh}{qh}",
                )
                r0 = wh * (C // 2) + qh * (C // 4)
                nc.sync.dma_start(
                    out=t,
                    in_=wpT[
                        r0 : r0 + C // 4,
                        fp * 256 : (fp + 1) * 256,
                    ].rearrange("(ct p) f -> p ct f", p=128),
                )
                return t

            def load_wp_pair(fp):
                # 4 chunk tiles per pair (2 C-halves x 2 ct-halves) so the
                # first matmul only gates on a quarter of the pair's bytes.
                return [
                    [load_wp_chunk(fp, wh, qh) for qh in range(2)]
                    for wh in range(2)
                ]

            # per-seq state kept across emit stages
            seq_state = {}
            hist_tiles = {}

            def load_hist(b, h):
                kt = p_hist.tile([128, hist], da, tag=f"kth{h}", name=f"kth{h}")
                nc.sync.dma_start(out=kt, in_=kTh[h, b])
                vt = p_hist.tile([128, NJH, 128], da, tag=f"vh{h}", name=f"vh{h}")
                nc.sync.dma_start(out=vt, in_=vh[h, b])
                hist_tiles[(b, h)] = (kt, vt)

            def qkv_steps(b):
                """Generator: emits QKV for sequence b, yielding after each
                of the 12 f-tiles so attention units of sequence b-1 can be
                interleaved into the PE stream."""
                st = {}
                seq_state[b] = st

                # Interleave the wp0 chunks with the hT chunks in transfer
                # order so the first f-tile's matmul chain starts after ~1MB
                # and chases the DMA stream instead of waiting for all of it.
                def load_hT(cc):
                    pool = p_hT if cc < 4 else p_hT1
                    t = pool.tile([128, HCT, Lq], dq, tag=f"hT{cc}", name=f"hT{cc}")
                    nc.sync.dma_start(
                        out=t,
                        in_=hT[
                            cc * HCT * 128 : (cc + 1) * HCT * 128,
                            b * Lq : (b + 1) * Lq,
                        ].rearrange("(ct p) t -> p ct t", p=128),
                    )
                    return t

                wp0 = [[None, None], [None, None]]
                hT_c = [None] * NHC
                wp0[0][0] = load_wp_chunk(0, 0, 0)
                hT_c[0] = load_hT(0)
                wp0[0][1] = load_wp_chunk(0, 0, 1)
                hT_c[1] = load_hT(1)
                hT_c[2] = load_hT(2)
                wp0[1][0] = load_wp_chunk(0, 1, 0)
                hT_c[3] = load_hT(3)
                if b == 0:
                    emit_small_consts()
                wp0[1][1] = load_wp_chunk(0, 1, 1)
                for cc in range(4, NHC):
                    hT_c[cc] = load_hT(cc)

                qrot = [None] * hpc
                krot = [None] * hpc
                vnat = [
                    p_vnat.tile([128, Fqk], da, tag=f"vnat{i}", name=f"vnat{i}")
                    for i in range(NJF)
                ]
                st.update(qrot=qrot, krot=krot, vnat=vnat)

                # epilogue of f-tile ft (RoPE or v-transposes), deferred by
                # one f-tile so the PE never stalls on the ScalarE eviction.
                def qkv_epilogue(ft, qs):
                    if ft < 2 * hpc:
                        pr = ps_rot.tile([128, Lq], F32, tag="rot", name="rot")
                        nc.tensor.matmul(pr, consts["rm16"], qs, start=True, stop=True)
                        tag = f"qrot{ft}" if ft < hpc else f"krot{ft - hpc}"
                        tmp1 = p_qs.tile([128, Lq], FP16, tag="tmp1", name="tmp1")
                        nc.vector.tensor_mul(tmp1, qs, consts["cos"])
                        tmp = p_qs.tile([128, Lq], FP16, tag="tmp", name="tmp")
                        nc.vector.tensor_mul(tmp, pr, consts["sin"])
                        dst = p_qk.tile([128, Lq], da, tag=tag)
                        nc.vector.tensor_add(dst, tmp1, tmp)
                        if ft < hpc:
                            qrot[ft] = dst
                        else:
                            krot[ft - hpc] = dst
                    else:
                        fv = ft - 2 * hpc
                        for tsub in range(NJF):
                            pt = ps_rot.tile([128, Lq], FP16, tag="rot", name="rot")
                            nc.tensor.transpose(
                                pt[:, 0:128],
                                qs[:, tsub * 128 : (tsub + 1) * 128],
                                consts["ident16"][:, :],
                            )
                            nc.vector.tensor_copy(
                                vnat[tsub][:, fv * 128 : (fv + 1) * 128],
                                pt[:, 0:128],
                            )

                st["epilogue"] = qkv_epilogue

                pending = None
                for fp in range(NFP):
                    wp_h = wp0 if fp == 0 else load_wp_pair(fp)
                    for sub in range(2):
                        ft = 2 * fp + sub
                        ps = ps_mm.tile([128, Lq], F32, tag="mm", name="mm")
                        for ct in range(NCT):
                            nc.tensor.matmul(
                                ps,
                                wp_h[ct // (NCT // 2)][(ct % (NCT // 2)) // (NCT // 4)][
                                    :, ct % (NCT // 4), sub * 128 : (sub + 1) * 128
                                ],
                                hT_c[ct // HCT][:, ct % HCT, :],
                                start=(ct == 0),
                                stop=(ct == NCT - 1),
                            )
                        qs = p_qs.tile([128, Lq], FP16, tag="qs", name="qs")
                        nc.scalar.copy(qs, ps)
                        if pending is not None:
                            qkv_epilogue(*pending)
                        pending = (ft, qs)
                        yield
                st["pending"] = pending

                # history K/V for this sequence. For b=0 issue now (behind
                # the wp/hT stream); for b>0 the loads were already issued
                # inside attn(b-1) as each head's tiles freed up (attn(b)
                # picks them out of hist_tiles lazily, per head).
                if b == 0:
                    for h in range(hpc):
                        load_hist(0, h)
                    emit_wo_load()

            def oproj_steps(b):
                """Generator of o_proj emission steps for sequence b.
                Each step: one (oc, tsub) group = hpc accumulating matmuls +
                fp16 eviction + output DMA. 32 steps total."""
                attnT = seq_state[b]["attnT"]
                for oc in range(NOC):
                    for tsub in range(NJF):
                        po = ps_mm.tile([128, 512], F32, tag="mm", name="mm")
                        for j in range(hpc):
                            nc.tensor.matmul(
                                po,
                                attnT[j][:, tsub * 128 : (tsub + 1) * 128],
                                consts["wo"][:, j, oc * 512 : (oc + 1) * 512],
                                start=(j == 0),
                                stop=(j == hpc - 1),
                            )
                        oe = p_oe.tile([128, 512], FP16, tag="oe", name="oe")
                        nc.vector.tensor_copy(oe, po)
                        row = b * Lq + tsub * 128
                        nc.sync.dma_start(
                            out=outp[row : row + 128, oc * 512 : (oc + 1) * 512],
                            in_=oe,
                        )
                        yield

            def attn_steps(b, op_iter):
                """Generator: attention for sequence b, yielding after each
                of the 64 units. o_proj steps of sequence b-1 (op_iter) are
                pulled internally every 2 units."""
                st = seq_state[b]
                qrot, krot, vnat = st["qrot"], st["krot"], st["vnat"]
                kth_t = [None] * hpc
                vh_tt = [None] * hpc
                attnT = [None] * hpc
                st["attnT"] = attnT
                P_t = {}
                pv_ps = {}
                den_acc = {}
                den_ps = {}
                actions = []   # (due_unit, fn), emitted after S/exp of a unit
                unit = 0
                pending_qkv = [st["pending"]]

                # unit order per head: history j=0..NJH-1 full width, then
                # fresh kv blocks in REVERSE order (jj=NJF-1 .. 0) with
                # partial q widths so the last fresh matmul is full-width and
                # carries the accumulation stop flag.
                def unit_j(u):
                    if u < NJH:
                        return u, 0           # j, q-offset
                    jj = NJF - 1 - (u - NJH)  # NJF-1 .. 0
                    return NJH + jj, jj * 128

                def emit_pv(h, u):
                    def fn():
                        j, qoff = unit_j(u)
                        pvh = pv_ps[h]
                        if j < NJH:
                            v_lhsT = vh_tt[h][:, j, :]
                        else:
                            v_lhsT = vnat[j - NJH][:, h * 128 : (h + 1) * 128]
                        P = P_t.pop((h, u))
                        nc.tensor.matmul(
                            pvh[:, qoff:Lq], v_lhsT, P[:, qoff:Lq],
                            start=(u == 0), stop=(u == NJ - 1),
                        )
                    return fn

                def emit_den_add(h, u):
                    def fn():
                        j, qoff = unit_j(u)
                        A = den_acc[h]
                        P = P_t[(h, u)]
                        if u == 0:
                            nc.vector.tensor_copy(A, P)
                        else:
                            nc.vector.tensor_add(
                                A[:, qoff:Lq], A[:, qoff:Lq], P[:, qoff:Lq]
                            )
                    return fn

                def emit_den_mm(h):
                    def fn():
                        dps = ps_rot.tile([128, Lq], F32, tag="rot", name="den")
                        den_ps[h] = dps
                        nc.tensor.matmul(
                            dps[0:1, :], consts["ones_col"], den_acc[h],
                            start=True, stop=True,
                        )
                        # this head's history tiles are fully consumed (last
                        # PV was emitted at due unit+6 < unit+7): start the
                        # next sequence's loads into the freed buffers.
                        if b + 1 < B:
                            load_hist(b + 1, h)
                    return fn

                def emit_norm(h):
                    def fn():
                        pvh = pv_ps[h]
                        recf = p_small.tile([1, Lq], F32, tag="recf", name="recf")
                        nc.vector.reciprocal_approx_fast(
                            out=recf, in_=den_ps[h][0:1, :]
                        )
                        recip = p_small.tile([1, Lq], da, tag="recip", name="recip")
                        nc.vector.tensor_copy(recip, recf)
                        bc = ps_rot.tile([128, Lq], F32, tag="rot", name="bc")
                        nc.tensor.matmul(
                            bc, consts["ones_row16"], recip, start=True, stop=True
                        )
                        bcs = p_small.tile([128, Lq], da, tag="bcs", name="bcs")
                        nc.vector.tensor_copy(bcs, bc)
                        at = p_attnT.tile(
                            [128, Lq], do, tag=f"attnT{h}", name=f"attnT{h}"
                        )
                        nc.vector.tensor_mul(at, pvh, bcs)
                        attnT[h] = at
                    return fn

                for h in range(hpc):
                    kth_t[h], vh_tt[h] = hist_tiles.pop((b, h))
                    pv_ps[h] = ps_pv.tile([128, Lq], F32, tag="pv", name="pv")
                    den_acc[h] = p_den.tile([128, Lq], da, tag="A", name="A")
                    for u in range(NJ):
                        j, qoff = unit_j(u)
                        w = Lq - qoff
                        sp = ps_s.tile([128, Lq], F32, tag="sps", name="sps")
                        if j < NJH:
                            k_lhsT = kth_t[h][:, j * 128 : (j + 1) * 128]
                        else:
                            jj = j - NJH
                            k_lhsT = krot[h][:, jj * 128 : (jj + 1) * 128]
                        nc.tensor.matmul(
                            sp[:, qoff:Lq], k_lhsT, qrot[h][:, qoff:Lq],
                            start=True, stop=True,
                        )
                        P = p_p.tile([128, Lq], da, tag="P", name="P")
                        if j < NJH:
                            nc.scalar.activation(
                                P, sp, mybir.ActivationFunctionType.Exp,
                                scale=scale, bias=consts["ebias"][:, :],
                            )
                        else:
                            # diagonal block: exp then triangular mask
                            Pf = p_pf.tile([128, 128], da, tag="Pf", name="Pf")
                            nc.scalar.activation(
                                Pf, sp[:, qoff : qoff + 128],
                                mybir.ActivationFunctionType.Exp,
                                scale=scale, bias=consts["ebias"][:, :],
                            )
                            nc.vector.tensor_mul(
                                P[:, qoff : qoff + 128], Pf, consts["tri"]
                            )
                            if qoff + 128 < Lq:
                                nc.scalar.activation(
                                    P[:, qoff + 128 : Lq], sp[:, qoff + 128 : Lq],
                                    mybir.ActivationFunctionType.Exp,
                                    scale=scale, bias=consts["ebias"][:, :],
                                )
                        P_t[(h, u)] = P
                        if pending_qkv and unit == 1:
                            st["epilogue"](*pending_qkv.pop())
                        actions.append((unit + 2, emit_den_add(h, u)))
                        actions.append(
                            (unit + (6 if j >= NJH else 3), emit_pv(h, u))
                        )
                        if u == NJ - 1:
                            actions.append((unit + 11, emit_den_mm(h)))
                            actions.append((unit + 13, emit_norm(h)))
                        unit += 1
                        while actions and actions[0][0] <= unit:
                            actions.pop(0)[1]()
                        if op_iter is not None and unit % 2 == 0:
                            next(op_iter, None)
                        yield
                while actions:
                    actions.pop(0)[1]()
                if op_iter is not None:
                    for _ in op_iter:
                        pass

            # Software pipeline: QKV(b+1)'s f-tiles are interleaved with
            # attention units of sequence b (which themselves pull o_proj
            # groups of sequence b-1), keeping the PE stream dense while
            # ScalarE works through the exps.
            _DONE = object()

            def drive():
                for _ in qkv_steps(0):
                    pass
                for b in range(B):
                    a = attn_steps(b, oproj_steps(b - 1) if b > 0 else None)
                    if b + 1 < B:
                        ftn = 0
                        for _ in qkv_steps(b + 1):
                            ftn += 1
                            pulls = 5 if ftn <= 8 else 6
                            for _ in range(pulls):
                                if next(a, _DONE) is _DONE:
                                    break
                    for _ in a:
                        pass
                for _ in oproj_steps(B - 1):
                    pass

            drive()
    nc.compile()
    return nc


def _np_dt(d):
    return mybir.dt.np(d)


def prepare_host_inputs(inputs):
    """Shard + relayout the full inputs into 8 per-core input maps."""
    hidden_states = np.ascontiguousarray(
        np.asarray(inputs["hidden_states"], np.float32)
    )
    w_pack = np.asarray(inputs["w_pack"], np.float32)
    w_o = np.asarray(inputs["w_o"], np.float32)
    k_cache = np.asarray(inputs["k_cache"], np.float32)
    v_cache = np.asarray(inputs["v_cache"], np.float32)
    block_offsets = np.asarray(inputs["block_offsets"])
    hist = int(inputs["history_len"])
    Lq = int(inputs["q_len"])
    bs = int(inputs["block_size"])

    B, nblk = block_offsets.shape
    H, D = k_cache.shape[2], k_cache.shape[3]
    hidden = H * D
    T = B * Lq
    assert hidden_states.shape == (T, hidden)
    assert hist % bs == 0 and Lq % bs == 0 and hist % 128 == 0
    hpc = H // N_CORES

    f16 = np.float16

    # shared tensors
    hT = np.ascontiguousarray(hidden_states.T).astype(f16)

    pos = hist + np.arange(Lq, dtype=np.float64)
    inv_freq = 1.0 / (10000.0 ** (np.arange(0, D, 2, dtype=np.float64) / D))
    ang = pos[None, :] * inv_freq[np.arange(D) % (D // 2), None]  # [D, Lq]
    cosT = np.ascontiguousarray(np.cos(ang)).astype(f16)
    sinT = np.ascontiguousarray(np.sin(ang)).astype(f16)

    Rm = np.zeros((D, D), np.float32)
    half = D // 2
    for d in range(half):
        Rm[d + half, d] = -1.0
    for d in range(half, D):
        Rm[d - half, d] = 1.0
    RmT = Rm.astype(f16)

    # [kv, q] diagonal-block causal mask: allow q >= kv
    triT = np.ascontiguousarray(np.triu(np.ones((128, 128), f16)))

    # paged gather of the history KV (host side = the sharding relayout)
    nhist_blk = hist // bs
    blocks_hist = block_offsets[:, :nhist_blk]
    k_hist = k_cache[blocks_hist].reshape(B, hist, H, D)
    v_hist = v_cache[blocks_hist].reshape(B, hist, H, D)
    NJH = hist // 128

    in_maps = []
    for c in range(N_CORES):
        hs = slice(c * hpc, (c + 1) * hpc)
        rows = np.concatenate(
            [
                q * hidden + np.arange(c * hpc * D, (c + 1) * hpc * D)
                for q in range(3)
            ]
        )
        wpT_c = np.ascontiguousarray(w_pack[rows].T).astype(f16)
        woT_c = np.ascontiguousarray(
            w_o[:, c * hpc * D : (c + 1) * hpc * D].T
        ).astype(f16)
        kTh_c = np.ascontiguousarray(
            k_hist[:, :, hs, :].transpose(2, 0, 3, 1)
        ).astype(f16)
        # v history pre-tiled: [h, b, p, j, d] with kv = j*128 + p
        vh_c = np.ascontiguousarray(
            v_hist[:, :, hs, :]
            .reshape(B, NJH, 128, hpc, D)
            .transpose(3, 0, 2, 1, 4)
        ).astype(f16)
        in_maps.append(
            {
                "hT": hT,
                "wpT": wpT_c,
                "woT": woT_c,
                "kTh": kTh_c,
                "vh": vh_c,
                "cosT": cosT,
                "sinT": sinT,
                "RmT": RmT,
                "triT": triT,
            }
        )
    meta = dict(B=B, Lq=Lq, H=H, D=D, hidden=hidden, hist=hist, hpc=hpc)
    return in_maps, meta


_NC_CACHE = {}


def get_nc(meta):
    key = tuple(sorted(meta.items()))
    if key not in _NC_CACHE:
        _NC_CACHE[key] = build_kernel(**meta)
    return _NC_CACHE[key]


def run(inputs, trace=False):
    in_maps, meta = prepare_host_inputs(inputs)
    nc = get_nc(meta)
    res = run_bass_kernel_spmd(nc, in_maps, list(range(N_CORES)), trace=trace)
    out = res.results[0]["outp"].astype(np.float32)
    for i in range(1, N_CORES):
        out += res.results[i]["outp"].astype(np.float32)
    return out, res


def kernel(**inputs):
    out, _ = run(inputs, trace=False)
    return out


# revision 38
# speedup vs baseline: 1.1927x; 1.0079x over previous
"""Trainium2 Bass kernel for paged-KV attention block (QKV proj + RoPE +
paged causal attention + o_proj), tensor-parallel over heads across 8 cores.

Contract: kernel(**inputs) takes the full unsharded inputs (numpy or jax
arrays, keyed as in the reference setup_inputs) and returns the full
[B*Lq, hidden] float32 output.

Sharding (per the tensor-parallel hint):
  - W_pack sharded over heads: each core owns 4 heads of q, k, v rows.
  - KV cache and attention sharded over the same heads.
  - o_proj row-sharded; each core computes a full [T, hidden] partial (fp16)
    and the partials are summed on the host (replaces the all-reduce).

Device schedule (v2):
  - QKV in transposed [feature, token] layout; fresh q/k land in the [d, t]
    layout scores need; v is PE-transposed back to [t, d] tiles.
  - K history pre-transposed on host to [h, b, d, kv]; V history pre-tiled
    to [h, b, p, j, d]; both DMA'd early in the QKV phase (not at attention
    start) so attention never waits on them.
  - w_o is hoisted to SBUF once for the whole kernel.
  - Scores as S^T [kv, q]; exp fused with PSUM eviction + 1/sqrt(D) scale on
    ScalarE. Causal structure exploited: fresh-kv tiles only compute the
    q >= kv columns; only the diagonal 128x128 block needs a mask multiply.
    Fresh PV/den accumulate in reverse kv order so the last (full-width)
    matmul carries the accumulation stop flag.
  - Softmax denominator: P tiles are accumulated on the (otherwise idle-ish)
    Vector engine into a per-head running sum; a single ones-vector matmul
    per head reduces it over partitions. This keeps the Tensor engine free
    of the 256 denominator matmuls and frees a PSUM bank.
  - o_proj of sequence b-1 is software-pipelined into the attention phase of
    sequence b (one [t-tile, 512-col] group every 2 attention units), filling
    the Tensor-engine bubbles that ScalarE's exp throughput would otherwise
    leave. The last sequence's o_proj runs at the end.
  - Output partials are written fp16 (host sums in fp32).
"""

import math

import numpy as np

import concourse.bacc as bacc
import concourse.tile as tile
from concourse import mybir
from concourse.bass_utils import run_bass_kernel_spmd

F32 = mybir.dt.float32
FP16 = mybir.dt.float16

N_CORES = 8


def build_kernel(B, Lq, H, D, hidden, hist, hpc):
    """Build the SPMD single-core program. hpc = heads per core."""
    assert D == 128 and Lq == 512 and hist % 128 == 0
    Fqk = hpc * D          # per-core q (or k) feature count = 512
    F3 = 3 * Fqk           # per-core packed qkv features = 1536
    T = B * Lq
    C = hidden
    NCT = C // 128         # contraction tiles = 32
    NJH = hist // 128      # kv tiles in history = 12
    NJF = Lq // 128        # kv tiles fresh = 4
    NJ = NJH + NJF         # 16
    NOC = hidden // 512    # o_proj column chunks = 8
    NFP = (3 * hpc) // 2   # wp 2-head pair loads per seq = 6
    scale = 1.0 / math.sqrt(D)
    EXP_BIAS = -8.0
    dq = FP16              # qkv matmul dtype
    da = FP16              # attention matmul dtype
    do = FP16              # o_proj matmul dtype

    nc = bacc.Bacc("TRN2")

    hT = nc.dram_tensor("hT", [C, T], dq, kind="ExternalInput")
    wpT = nc.dram_tensor("wpT", [C, F3], dq, kind="ExternalInput")
    woT = nc.dram_tensor("woT", [Fqk, hidden], do, kind="ExternalInput")
    kTh = nc.dram_tensor("kTh", [hpc, B, D, hist], da, kind="ExternalInput")
    vh = nc.dram_tensor("vh", [hpc, B, 128, NJH, 128], da, kind="ExternalInput")
    cosT = nc.dram_tensor("cosT", [D, Lq], FP16, kind="ExternalInput")
    sinT = nc.dram_tensor("sinT", [D, Lq], FP16, kind="ExternalInput")
    triT = nc.dram_tensor("triT", [128, 128], FP16, kind="ExternalInput")
    outp = nc.dram_tensor("outp", [T, hidden], FP16, kind="ExternalOutput")

    NHC = 8                # hT DMA chunks per seq
    HCT = NCT // NHC       # c-tiles per hT chunk = 4
    with tile.TileContext(nc) as tc:
        with (
            tc.tile_pool(name="const", bufs=1) as p_const,
            tc.tile_pool(name="hTp", bufs=2) as p_hT,
            tc.tile_pool(name="hTp1", bufs=1) as p_hT1,
            tc.tile_pool(name="wpp", bufs=2) as p_wp,
            tc.tile_pool(name="qsp", bufs=2) as p_qs,
            tc.tile_pool(name="qk", bufs=2) as p_qk,
            tc.tile_pool(name="vnatp", bufs=2) as p_vnat,
            tc.tile_pool(name="attnTp", bufs=2) as p_attnT,
            tc.tile_pool(name="hist", bufs=1) as p_hist,
            tc.tile_pool(name="Pp", bufs=10) as p_p,
            tc.tile_pool(name="Pfp", bufs=3) as p_pf,
            tc.tile_pool(name="denp", bufs=2) as p_den,
            tc.tile_pool(name="smalls", bufs=2) as p_small,
            tc.tile_pool(name="oep", bufs=3) as p_oe,
            tc.tile_pool(name="ps_mm", bufs=2, space="PSUM") as ps_mm,
            tc.tile_pool(name="ps_rot", bufs=2, space="PSUM") as ps_rot,
            tc.tile_pool(name="ps_s", bufs=2, space="PSUM") as ps_s,
            tc.tile_pool(name="ps_pv", bufs=2, space="PSUM") as ps_pv,
        ):
            consts = {}

            def emit_small_consts():
                cos_sb = p_const.tile([D, Lq], FP16, tag="cos", name="cos")
                nc.sync.dma_start(out=cos_sb, in_=cosT[:, :])
                sin_sb = p_const.tile([D, Lq], FP16, tag="sin", name="sin")
                nc.sync.dma_start(out=sin_sb, in_=sinT[:, :])
                tri = p_const.tile([128, 128], FP16, tag="tri", name="tri")
                nc.sync.dma_start(out=tri, in_=triT[:, :])
                ident_sb = p_const.tile([128, 128], F32, tag="ident", name="ident")
                from concourse.masks import make_identity

                make_identity(nc, ident_sb[:, :])
                ident16 = p_const.tile([128, 128], FP16, tag="ident16", name="ident16")
                nc.vector.tensor_copy(ident16, ident_sb)
                ones_f32 = p_const.tile([128, 1], F32, tag="ones_f32", name="ones_f32")
                nc.vector.memset(ones_f32, 1.0)
                ones_col = p_const.tile([128, 1], da, tag="ones_col", name="ones_col")
                nc.vector.tensor_copy(ones_col, ones_f32)
                ebias_sb = p_const.tile([128, 1], F32, tag="ebias", name="ebias")
                nc.vector.memset(ebias_sb, EXP_BIAS)
                consts.update(
                    cos=cos_sb, sin=sin_sb, tri=tri, ident16=ident16,
                    ones_col=ones_col, ebias=ebias_sb,
                )

            def emit_wo_load():
                wo_sb = p_const.tile([128, hpc, hidden], do, tag="wo", name="wo")
                nc.sync.dma_start(
                    out=wo_sb,
                    in_=woT[:, :].rearrange("(jt p) o -> p jt o", p=128),
                )
                consts["wo"] = wo_sb

            def load_wp_chunk(fp, wh, qh):
                t = p_wp.tile(
                    [128, NCT // 4, 256], dq,
                    tag=f"wp{wh}{qh}", name=f"wp{wh}{qh}",
                )
                r0 = wh * (C // 2) + qh * (C // 4)
                nc.sync.dma_start(
                    out=t,
                    in_=wpT[
                        r0 : r0 + C // 4,
                        fp * 256 : (fp + 1) * 256,
                    ].rearrange("(ct p) f -> p ct f", p=128),
                )
                return t

            def load_wp_pair(fp):
                # 4 chunk tiles per pair (2 C-halves x 2 ct-halves) so the
                # first matmul only gates on a quarter of the pair's bytes.
                return [
                    [load_wp_chunk(fp, wh, qh) for qh in range(2)]
                    for wh in range(2)
                ]

            # per-seq state kept across emit stages
            seq_state = {}
            hist_tiles = {}

            def load_hist(b, h):
                kt = p_hist.tile([128, hist], da, tag=f"kth{h}", name=f"kth{h}")
                nc.sync.dma_start(out=kt, in_=kTh[h, b])
                vt = p_hist.tile([128, NJH, 128], da, tag=f"vh{h}", name=f"vh{h}")
                nc.sync.dma_start(out=vt, in_=vh[h, b])
                hist_tiles[(b, h)] = (kt, vt)

            def qkv_steps(b):
                """Generator: emits QKV for sequence b, yielding after each
                of the 12 f-tiles so attention units of sequence b-1 can be
                interleaved into the PE stream."""
                st = {}
                seq_state[b] = st

                # Interleave the wp0 chunks with the hT chunks in transfer
                # order so the first f-tile's matmul chain starts after ~1MB
                # and chases the DMA stream instead of waiting for all of it.
                def load_hT(cc):
                    pool = p_hT if cc < 4 else p_hT1
                    t = pool.tile([128, HCT, Lq], dq, tag=f"hT{cc}", name=f"hT{cc}")
                    nc.sync.dma_start(
                        out=t,
                        in_=hT[
                            cc * HCT * 128 : (cc + 1) * HCT * 128,
                            b * Lq : (b + 1) * Lq,
                        ].rearrange("(ct p) t -> p ct t", p=128),
                    )
                    return t

                wp0 = [[None, None], [None, None]]
                hT_c = [None] * NHC
                wp0[0][0] = load_wp_chunk(0, 0, 0)
                hT_c[0] = load_hT(0)
                wp0[0][1] = load_wp_chunk(0, 0, 1)
                hT_c[1] = load_hT(1)
                hT_c[2] = load_hT(2)
                wp0[1][0] = load_wp_chunk(0, 1, 0)
                hT_c[3] = load_hT(3)
                if b == 0:
                    emit_small_consts()
                wp0[1][1] = load_wp_chunk(0, 1, 1)
                for cc in range(4, NHC):
                    hT_c[cc] = load_hT(cc)

                qrot = [None] * hpc
                krot = [None] * hpc
                vnat = [
                    p_vnat.tile([128, Fqk], da, tag=f"vnat{i}", name=f"vnat{i}")
                    for i in range(NJF)
                ]
                st.update(qrot=qrot, krot=krot, vnat=vnat)

                # epilogue of f-tile ft (RoPE or v-transposes), deferred by
                # one f-tile so the PE never stalls on the ScalarE eviction.
                # rotate_half is a pure partition swap: done with two
                # SBUF->SBUF DMAs (sign of the lower half is folded into the
                # sin constant) instead of a PE matmul.
                def qkv_epilogue(ft, qs):
                    if ft < 2 * hpc:
                        tag = f"qrot{ft}" if ft < hpc else f"krot{ft - hpc}"
                        qsw = p_qs.tile([128, Lq], FP16, tag="qsw", name="qsw")
                        half = D // 2
                        nc.sync.dma_start(out=qsw[0:half, :], in_=qs[half:D, :])
                        nc.sync.dma_start(out=qsw[half:D, :], in_=qs[0:half, :])
                        tmp1 = p_qs.tile([128, Lq], FP16, tag="tmp1", name="tmp1")
                        nc.vector.tensor_mul(tmp1, qs, consts["cos"])
                        nc.vector.tensor_mul(qsw, qsw, consts["sin"])
                        dst = p_qk.tile([128, Lq], da, tag=tag)
                        nc.vector.tensor_add(dst, tmp1, qsw)
                        if ft < hpc:
                            qrot[ft] = dst
                        else:
                            krot[ft - hpc] = dst
                    else:
                        fv = ft - 2 * hpc
                        for tsub in range(NJF):
                            pt = ps_rot.tile([128, Lq], FP16, tag="rot", name="rot")
                            nc.tensor.transpose(
                                pt[:, 0:128],
                                qs[:, tsub * 128 : (tsub + 1) * 128],
                                consts["ident16"][:, :],
                            )
                            nc.vector.tensor_copy(
                                vnat[tsub][:, fv * 128 : (fv + 1) * 128],
                                pt[:, 0:128],
                            )

                st["epilogue"] = qkv_epilogue

                pending = None
                for fp in range(NFP):
                    wp_h = wp0 if fp == 0 else load_wp_pair(fp)
                    for sub in range(2):
                        ft = 2 * fp + sub
                        ps = ps_mm.tile([128, Lq], F32, tag="mm", name="mm")
                        for ct in range(NCT):
                            nc.tensor.matmul(
                                ps,
                                wp_h[ct // (NCT // 2)][(ct % (NCT // 2)) // (NCT // 4)][
                                    :, ct % (NCT // 4), sub * 128 : (sub + 1) * 128
                                ],
                                hT_c[ct // HCT][:, ct % HCT, :],
                                start=(ct == 0),
                                stop=(ct == NCT - 1),
                            )
                        qs = p_qs.tile([128, Lq], FP16, tag="qs", name="qs")
                        nc.scalar.copy(qs, ps)
                        if pending is not None:
                            qkv_epilogue(*pending)
                        pending = (ft, qs)
                        yield
                st["pending"] = pending

                # history K/V for this sequence. For b=0 issue now (behind
                # the wp/hT stream); for b>0 the loads were already issued
                # inside attn(b-1) as each head's tiles freed up (attn(b)
                # picks them out of hist_tiles lazily, per head).
                if b == 0:
                    for h in range(hpc):
                        load_hist(0, h)
                    emit_wo_load()

            def oproj_steps(b):
                """Generator of o_proj emission steps for sequence b.
                Each step: one (oc, tsub) group = hpc accumulating matmuls +
                fp16 eviction + output DMA. 32 steps total."""
                attnT = seq_state[b]["attnT"]
                for oc in range(NOC):
                    for tsub in range(NJF):
                        po = ps_mm.tile([128, 512], F32, tag="mm", name="mm")
                        for j in range(hpc):
                            nc.tensor.matmul(
                                po,
                                attnT[j][:, tsub * 128 : (tsub + 1) * 128],
                                consts["wo"][:, j, oc * 512 : (oc + 1) * 512],
                                start=(j == 0),
                                stop=(j == hpc - 1),
                            )
                        oe = p_oe.tile([128, 512], FP16, tag="oe", name="oe")
                        nc.vector.tensor_copy(oe, po)
                        row = b * Lq + tsub * 128
                        nc.sync.dma_start(
                            out=outp[row : row + 128, oc * 512 : (oc + 1) * 512],
                            in_=oe,
                        )
                        yield

            def attn_steps(b, op_iter):
                """Generator: attention for sequence b, yielding after each
                of the 64 units. o_proj steps of sequence b-1 (op_iter) are
                pulled internally every 2 units."""
                st = seq_state[b]
                qrot, krot, vnat = st["qrot"], st["krot"], st["vnat"]
                kth_t = [None] * hpc
                vh_tt = [None] * hpc
                attnT = [None] * hpc
                st["attnT"] = attnT
                P_t = {}
                pv_ps = {}
                den_acc = {}
                den_ps = {}
                actions = []   # (due_unit, fn), emitted after S/exp of a unit
                unit = 0
                pending_qkv = [st["pending"]]

                # unit order per head: history j=0..NJH-1 full width, then
                # fresh kv blocks in REVERSE order (jj=NJF-1 .. 0) with
                # partial q widths so the last fresh matmul is full-width and
                # carries the accumulation stop flag.
                def unit_j(u):
                    if u < NJH:
                        return u, 0           # j, q-offset
                    jj = NJF - 1 - (u - NJH)  # NJF-1 .. 0
                    return NJH + jj, jj * 128

                def emit_pv(h, u):
                    def fn():
                        j, qoff = unit_j(u)
                        pvh = pv_ps[h]
                        if j < NJH:
                            v_lhsT = vh_tt[h][:, j, :]
                        else:
                            v_lhsT = vnat[j - NJH][:, h * 128 : (h + 1) * 128]
                        P = P_t.pop((h, u))
                        nc.tensor.matmul(
                            pvh[:, qoff:Lq], v_lhsT, P[:, qoff:Lq],
                            start=(u == 0), stop=(u == NJ - 1),
                        )
                    return fn

                def emit_den_add(h, u):
                    def fn():
                        j, qoff = unit_j(u)
                        A = den_acc[h]
                        P = P_t[(h, u)]
                        if u == 0:
                            nc.vector.tensor_copy(A, P)
                        else:
                            nc.vector.tensor_add(
                                A[:, qoff:Lq], A[:, qoff:Lq], P[:, qoff:Lq]
                            )
                    return fn

                def emit_den_mm(h):
                    def fn():
                        dps = ps_rot.tile([128, Lq], F32, tag="rot", name="den")
                        den_ps[h] = dps
                        nc.tensor.matmul(
                            dps[0:1, :], consts["ones_col"], den_acc[h],
                            start=True, stop=True,
                        )
                        # this head's history tiles are fully consumed (last
                        # PV was emitted at due unit+6 < unit+7): start the
                        # next sequence's loads into the freed buffers.
                        if b + 1 < B:
                            load_hist(b + 1, h)
                    return fn

                def emit_norm(h):
                    def fn():
                        pvh = pv_ps[h]
                        recf = p_small.tile([1, Lq], F32, tag="recf", name="recf")
                        nc.vector.reciprocal_approx_fast(
                            out=recf, in_=den_ps[h][0:1, :]
                        )
                        recip = p_small.tile([1, Lq], da, tag="recip", name="recip")
                        nc.vector.tensor_copy(recip, recf)
                        bcs = p_small.tile([128, Lq], da, tag="bcs", name="bcs")
                        nc.gpsimd.partition_broadcast(bcs, recip)
                        at = p_attnT.tile(
                            [128, Lq], do, tag=f"attnT{h}", name=f"attnT{h}"
                        )
                        nc.vector.tensor_mul(at, pvh, bcs)
                        attnT[h] = at
                    return fn

                for h in range(hpc):
                    kth_t[h], vh_tt[h] = hist_tiles.pop((b, h))
                    pv_ps[h] = ps_pv.tile([128, Lq], F32, tag="pv", name="pv")
                    den_acc[h] = p_den.tile([128, Lq], da, tag="A", name="A")
                    for u in range(NJ):
                        j, qoff = unit_j(u)
                        w = Lq - qoff
                        sp = ps_s.tile([128, Lq], F32, tag="sps", name="sps")
                        if j < NJH:
                            k_lhsT = kth_t[h][:, j * 128 : (j + 1) * 128]
                        else:
                            jj = j - NJH
                            k_lhsT = krot[h][:, jj * 128 : (jj + 1) * 128]
                        nc.tensor.matmul(
                            sp[:, qoff:Lq], k_lhsT, qrot[h][:, qoff:Lq],
                            start=True, stop=True,
                        )
                        P = p_p.tile([128, Lq], da, tag="P", name="P")
                        if j < NJH:
                            nc.scalar.activation(
                                P, sp, mybir.ActivationFunctionType.Exp,
                                scale=scale, bias=consts["ebias"][:, :],
                            )
                        else:
                            # diagonal block: exp then triangular mask
                            Pf = p_pf.tile([128, 128], da, tag="Pf", name="Pf")
                            nc.scalar.activation(
                                Pf, sp[:, qoff : qoff + 128],
                                mybir.ActivationFunctionType.Exp,
                                scale=scale, bias=consts["ebias"][:, :],
                            )
                            nc.vector.tensor_mul(
                                P[:, qoff : qoff + 128], Pf, consts["tri"]
                            )
                            if qoff + 128 < Lq:
                                nc.scalar.activation(
                                    P[:, qoff + 128 : Lq], sp[:, qoff + 128 : Lq],
                                    mybir.ActivationFunctionType.Exp,
                                    scale=scale, bias=consts["ebias"][:, :],
                                )
                        P_t[(h, u)] = P
                        if pending_qkv and unit == 1:
                            st["epilogue"](*pending_qkv.pop())
                        actions.append((unit + 2, emit_den_add(h, u)))
                        actions.append(
                            (unit + (6 if j >= NJH else 3), emit_pv(h, u))
                        )
                        if u == NJ - 1:
                            # last sequence: no following QKV phase hides the
                            # norm chain, so schedule it tighter.
                            dd, nd = (8, 10) if b == B - 1 else (11, 13)
                            actions.append((unit + dd, emit_den_mm(h)))
                            actions.append((unit + nd, emit_norm(h)))
                        unit += 1
                        while actions and actions[0][0] <= unit:
                            actions.pop(0)[1]()
                        if op_iter is not None and unit % 2 == 0:
                            next(op_iter, None)
                        yield
                while actions:
                    actions.pop(0)[1]()
                if op_iter is not None:
                    for _ in op_iter:
                        pass

            # Software pipeline: QKV(b+1)'s f-tiles are interleaved with
            # attention units of sequence b (which themselves pull o_proj
            # groups of sequence b-1), keeping the PE stream dense while
            # ScalarE works through the exps.
            _DONE = object()

            def drive():
                for _ in qkv_steps(0):
                    pass
                for b in range(B):
                    a = attn_steps(b, oproj_steps(b - 1) if b > 0 else None)
                    if b + 1 < B:
                        ftn = 0
                        for _ in qkv_steps(b + 1):
                            ftn += 1
                            pulls = 5 if ftn <= 8 else 6
                            for _ in range(pulls):
                                if next(a, _DONE) is _DONE:
                                    break
                    for _ in a:
                        pass
                for _ in oproj_steps(B - 1):
                    pass

            drive()
    nc.compile()
    return nc


def prepare_host_inputs(inputs):
    """Shard + relayout the full inputs into 8 per-core input maps."""
    hidden_states = np.ascontiguousarray(
        np.asarray(inputs["hidden_states"], np.float32)
    )
    w_pack = np.asarray(inputs["w_pack"], np.float32)
    w_o = np.asarray(inputs["w_o"], np.float32)
    k_cache = np.asarray(inputs["k_cache"], np.float32)
    v_cache = np.asarray(inputs["v_cache"], np.float32)
    block_offsets = np.asarray(inputs["block_offsets"])
    hist = int(inputs["history_len"])
    Lq = int(inputs["q_len"])
    bs = int(inputs["block_size"])

    B, nblk = block_offsets.shape
    H, D = k_cache.shape[2], k_cache.shape[3]
    hidden = H * D
    T = B * Lq
    assert hidden_states.shape == (T, hidden)
    assert hist % bs == 0 and Lq % bs == 0 and hist % 128 == 0
    hpc = H // N_CORES

    f16 = np.float16

    # shared tensors
    hT = np.ascontiguousarray(hidden_states.T).astype(f16)

    pos = hist + np.arange(Lq, dtype=np.float64)
    inv_freq = 1.0 / (10000.0 ** (np.arange(0, D, 2, dtype=np.float64) / D))
    ang = pos[None, :] * inv_freq[np.arange(D) % (D // 2), None]  # [D, Lq]
    cosT = np.ascontiguousarray(np.cos(ang)).astype(f16)
    # rotate_half sign folded into sin: rows < D/2 multiply the swapped-in
    # upper half with a minus sign.
    sgn = np.where(np.arange(D) < D // 2, -1.0, 1.0)[:, None]
    sinT = np.ascontiguousarray(sgn * np.sin(ang)).astype(f16)

    # [kv, q] diagonal-block causal mask: allow q >= kv
    triT = np.ascontiguousarray(np.triu(np.ones((128, 128), f16)))

    # paged gather of the history KV (host side = the sharding relayout)
    nhist_blk = hist // bs
    blocks_hist = block_offsets[:, :nhist_blk]
    k_hist = k_cache[blocks_hist].reshape(B, hist, H, D)
    v_hist = v_cache[blocks_hist].reshape(B, hist, H, D)
    NJH = hist // 128

    in_maps = []
    for c in range(N_CORES):
        hs = slice(c * hpc, (c + 1) * hpc)
        rows = np.concatenate(
            [
                q * hidden + np.arange(c * hpc * D, (c + 1) * hpc * D)
                for q in range(3)
            ]
        )
        wpT_c = np.ascontiguousarray(w_pack[rows].T).astype(f16)
        woT_c = np.ascontiguousarray(
            w_o[:, c * hpc * D : (c + 1) * hpc * D].T
        ).astype(f16)
        kTh_c = np.ascontiguousarray(
            k_hist[:, :, hs, :].transpose(2, 0, 3, 1)
        ).astype(f16)
        # v history pre-tiled: [h, b, p, j, d] with kv = j*128 + p
        vh_c = np.ascontiguousarray(
            v_hist[:, :, hs, :]
            .reshape(B, NJH, 128, hpc, D)
            .transpose(3, 0, 2, 1, 4)
        ).astype(f16)
        in_maps.append(
            {
                "hT": hT,
                "wpT": wpT_c,
                "woT": woT_c,
                "kTh": kTh_c,
                "vh": vh_c,
                "cosT": cosT,
                "sinT": sinT,
                "triT": triT,
            }
        )
    meta = dict(B=B, Lq=Lq, H=H, D=D, hidden=hidden, hist=hist, hpc=hpc)
    return in_maps, meta


_NC_CACHE = {}


def get_nc(meta):
    key = tuple(sorted(meta.items()))
    if key not in _NC_CACHE:
        _NC_CACHE[key] = build_kernel(**meta)
    return _NC_CACHE[key]


def run(inputs, trace=False):
    in_maps, meta = prepare_host_inputs(inputs)
    nc = get_nc(meta)
    res = run_bass_kernel_spmd(nc, in_maps, list(range(N_CORES)), trace=trace)
    out = res.results[0]["outp"].astype(np.float32)
    for i in range(1, N_CORES):
        out += res.results[i]["outp"].astype(np.float32)
    return out, res


def kernel(**inputs):
    out, _ = run(inputs, trace=False)
    return out
